# revision 3
# baseline (speedup 1.0000x reference)
"""Partial-FC conv classifier kernel for 8 TRN2 NeuronCores.

Problem (hardcoded shapes): x [512, 512, 7, 7] f32, labels [512] i64,
weight [85742, 512, 1, 1] f32, bias [85742] f32.
reference: labels_unique = unique(labels, size=512, fill=0); w_sub =
weight[labels_unique]; logits = conv1x1(x, w_sub) + b_sub -> [512, 512, 7, 7].

Strategy: the unique-label gather is host-side data staging (it selects
512 rows / 1MB out of the 176MB table). The conv1x1 is a matmul
  out[u, (b,s)] = sum_c w_sub[u, c] * x[b, c, s].
Data-parallel over batch: core i computes batches [64*i, 64*(i+1)) with the
gathered weight replicated. Per core: [512x512] @ [512x3136] in fp16 with
fp32 PSUM accumulation (fp8 measures 4e-2 rel err -- fails the 2e-2 gate --
so fp16's ~21us PE floor is the compute roofline here).

Trace-derived hardware model driving this schedule:
  - framework preamble ends ~6.3us; nothing (DMA or matmul) can start
    before it.
  - each HWDGE queue (Sync, Scalar) dispatches roughly one DMA packet per
    ~10-20ns; a packet covers one SBUF partition's contiguous run, so
    per-partition run length IS the packet size. x therefore moves in
    [128 x 1568-col] units (3136B packets) from per-unit-contiguous host
    arrays; small strided slices (sub-2KB runs) would halve the wire rate.
  - the PE HAM clock-gate needs ~3.5us of continuous matmul activity to
    reach full rate and drops after ~1-2us of idle, so warm-up matmuls
    start immediately and the real matmul stream must never gap.
  - cols 0:1568 run k-OUTER in 392-col sections (each stage gated on one
    (w-half, x-unit) pair as it lands); cols 1568:3136 run k-inner/m-outer
    (x resident by then) so outputs drain per-m and the tail after the
    last matmul is just one 100KB piece.
"""

import numpy as np

import concourse.bass as bass  # noqa: F401  (registers types)
import concourse.mybir as mybir
import concourse.tile as tile
from concourse import bacc
from concourse.bass_utils import run_bass_kernel_spmd

N_CORES = 8
B = 512          # batch
C = 512          # channels (contraction)
HW = 49          # 7*7 spatial
U = 512          # unique labels (all distinct by construction)
B_LOC = B // N_CORES      # 64 batches per core
N_LOC = B_LOC * HW        # 3136 moving-dim columns per core
KT = C // 128             # 4 contraction tiles
MT = U // 128             # 4 output-partition tiles
XU = 1568                 # x DMA unit width (3136B per-partition packets)
SEC = 392                 # section width (one PSUM bank at fp32)
NSEC = N_LOC // SEC       # 8 sections
KO_SECS = XU // SEC       # sections 0..3 (cols 0:1568) run k-outer
N_WARM = 5                # warm-up matmuls bridging preamble -> first data

F32 = mybir.dt.float32
F16 = mybir.dt.float16

_MODULE = None


def _build_module():
    nc = bacc.Bacc("TRN2", target_bir_lowering=False, debug=False)
    # Every (slice) DMA unit below is a fully contiguous block of its host
    # array: packets span the whole per-partition run.
    xh = nc.dram_tensor("xh", [KT, 2, 128, XU], F16, kind="ExternalInput").ap()
    wT = nc.dram_tensor("wT", [2, 128, 2, U], F16, kind="ExternalInput").ap()
    bs = nc.dram_tensor("bs", [128, MT], F32, kind="ExternalInput").ap()
    out = nc.dram_tensor("out", [U, N_LOC], F16, kind="ExternalOutput").ap()

    with tile.TileContext(nc) as tc:
        with (
            tc.tile_pool(name="wpool", bufs=2) as wpool,
            tc.tile_pool(name="bpool", bufs=1) as bpool,
            tc.tile_pool(name="scr", bufs=1) as scr,
            tc.tile_pool(name="xpool", bufs=2 * KT) as xpool,
            tc.tile_pool(name="opool", bufs=MT) as opool,
            tc.tile_pool(name="psum", bufs=8, space="PSUM") as psum,
        ):
            # ---- input stream. Queue plan (first matmul is gated on
            # wA + xh(0,0); stage order in section 0 is k0,k2,k1,k3 to
            # match these arrival times):
            #   Sync:   wA(k01), wB(k23), bias, xh(1,0), xh(3,0), xh(1,1), xh(3,1)
            #   Scalar: xh(0,0), xh(2,0), xh(0,1), xh(2,1)
            w_sb = [wpool.tile([128, 2, U], F16, tag="w", name=f"w_{g}")
                    for g in range(2)]
            xh_sb = [[xpool.tile([128, XU], F16, tag="x", name=f"x_{k}_{h}")
                      for h in range(2)] for k in range(KT)]
            b_sb = bpool.tile([128, MT], F32)

            nc.sync.dma_start(w_sb[0][:], wT[0])
            nc.scalar.dma_start(xh_sb[0][0][:], xh[0, 0])
            nc.sync.dma_start(w_sb[1][:], wT[1])
            nc.scalar.dma_start(xh_sb[2][0][:], xh[2, 0])
            nc.sync.dma_start(b_sb[:], bs[:])
            nc.scalar.dma_start(xh_sb[0][1][:], xh[0, 1])
            nc.sync.dma_start(xh_sb[1][0][:], xh[1, 0])
            nc.scalar.dma_start(xh_sb[2][1][:], xh[2, 1])
            nc.sync.dma_start(xh_sb[3][0][:], xh[3, 0])
            nc.sync.dma_start(xh_sb[1][1][:], xh[1, 1])
            nc.sync.dma_start(xh_sb[3][1][:], xh[3, 1])

            # Warm-ups: keep the PE busy (and the HAM clock-gate ramping)
            # from the moment the preamble ends until real data lands.
            scr_sb = scr.tile([128, 512], F16)
            nc.gpsimd.memset(scr_sb[:], 0.0)
            for i in range(N_WARM):
                ps_warm = psum.tile([128, SEC], F32, tag="ps", name=f"warm_{i}")
                nc.tensor.matmul(
                    ps_warm[:], scr_sb[:, :128], scr_sb[:, 120:512],
                    start=True, stop=True,
                )

            # Output staging: one full row-block per m-tile
            o_sb = [opool.tile([128, N_LOC], F16, tag="o", name=f"o_{m}")
                    for m in range(MT)]

            def w_slice(k, m):
                return w_sb[k // 2][:, k % 2, m * 128:(m + 1) * 128]

            def x_slice(k, c0, c1):
                h = c0 // XU
                return xh_sb[k][h][:, c0 - h * XU:c1 - h * XU]

            def evict(ps, m, c0, c1, eng):
                dst = o_sb[m][:, c0:c1]
                if eng == "s":
                    nc.scalar.activation(
                        dst, ps[:], mybir.ActivationFunctionType.Identity,
                        bias=b_sb[:, m:m + 1],
                    )
                else:
                    nc.vector.tensor_scalar_add(dst, ps[:], b_sb[:, m:m + 1])

            # ---- sections 0..3 (cols 0:1568): k-outer, each stage gated
            # on one (w-half, x-unit) pair; 4 PSUM banks per section.
            K_ORDER = (0, 2, 1, 3)  # matches DMA arrival order
            for s in range(KO_SECS):
                c0, c1 = s * SEC, (s + 1) * SEC
                ps_s = [psum.tile([128, SEC], F32, tag="ps",
                                  name=f"ps_{s}_{m}") for m in range(MT)]
                korder = K_ORDER if s == 0 else range(KT)
                for ki, k in enumerate(korder):
                    xs = x_slice(k, c0, c1)
                    for m in range(MT):
                        nc.tensor.matmul(
                            ps_s[m][:], w_slice(k, m), xs,
                            start=(ki == 0), stop=(ki == KT - 1),
                        )
                for m in range(MT):
                    eng = "v" if (s == 0 or m % 2 == 0) else "s"
                    evict(ps_s[m], m, c0, c1, eng)
                if s == KO_SECS - 1:
                    # cols 0:1568 of each row block complete -> first piece
                    for m in range(MT):
                        nc.sync.dma_start(out[m * 128:(m + 1) * 128, 0:XU],
                                          o_sb[m][:, 0:XU])

            # ---- sections 4..7 (cols 1568:3136): x is resident; k-inner
            # per m so outputs drain per-m with a short tail.
            for s in range(KO_SECS, NSEC):
                c0, c1 = s * SEC, (s + 1) * SEC
                for m in range(MT):
                    ps = psum.tile([128, SEC], F32, tag="ps",
                                   name=f"ps_{s}_{m}")
                    for k in range(KT):
                        nc.tensor.matmul(
                            ps[:], w_slice(k, m), x_slice(k, c0, c1),
                            start=(k == 0), stop=(k == KT - 1),
                        )
                    eng = "v" if m % 2 == 0 else "s"
                    evict(ps, m, c0, c1, eng)
                    if s == NSEC - 1:
                        # last section: drain per-m right after its evict
                        dma_eng = nc.scalar if eng == "s" else nc.sync
                        dma_eng.dma_start(
                            out[m * 128:(m + 1) * 128, 2744:3136],
                            o_sb[m][:, 2744:3136],
                        )
                if s == NSEC - 2:
                    # cols 1568:2744 complete per row block -> second piece
                    for m in range(MT):
                        nc.sync.dma_start(
                            out[m * 128:(m + 1) * 128, XU:2744],
                            o_sb[m][:, XU:2744],
                        )

    nc.compile()
    return nc


def _get_module():
    global _MODULE
    if _MODULE is None:
        _MODULE = _build_module()
    return _MODULE


def _prep_inputs(x, labels, weight, bias):
    x = np.asarray(x)
    labels = np.asarray(labels)
    weight = np.asarray(weight)
    bias = np.asarray(bias, dtype=np.float32)

    # jnp.unique(labels, size=B, fill_value=0): sorted unique, padded with 0.
    u = np.unique(labels)
    if u.size < U:
        u = np.concatenate([u, np.zeros(U - u.size, dtype=u.dtype)])
    u = u[:U]

    w_sub = weight.reshape(weight.shape[0], C)[u]                    # [U, C]
    # wT[g, p, j, m] = w_sub[m, (2g+j)*128+p]
    wT = np.ascontiguousarray(
        w_sub.T.astype(np.float16).reshape(2, 2, 128, U).transpose(0, 2, 1, 3)
    )
    b_sub = np.ascontiguousarray(bias[u].reshape(MT, 128).T)         # [128, MT]

    x16 = x.reshape(B, C, HW).astype(np.float16)
    in_maps = []
    for i in range(N_CORES):
        xi = x16[i * B_LOC:(i + 1) * B_LOC]
        # c = t*128+p, col = b*49+s; unit (k, h) is a contiguous [128, XU]
        xt = (
            xi.transpose(1, 0, 2).reshape(KT, 128, 2, XU).transpose(0, 2, 1, 3)
        )
        in_maps.append({
            "xh": np.ascontiguousarray(xt), "wT": wT, "bs": b_sub,
        })
    return in_maps


def _assemble_output(results):
    parts = []
    for i in range(N_CORES):
        oi = np.asarray(results[i]["out"]).astype(np.float32)  # [U, N_LOC]
        parts.append(
            np.ascontiguousarray(
                oi.reshape(U, B_LOC, HW).transpose(1, 0, 2)
            ).reshape(B_LOC, U, 7, 7)
        )
    return np.concatenate(parts, axis=0)


def run(x, labels, weight, bias, trace=False):
    in_maps = _prep_inputs(x, labels, weight, bias)
    nc = _get_module()
    res = run_bass_kernel_spmd(
        nc, in_maps, core_ids=list(range(N_CORES)), trace=trace
    )
    return _assemble_output(res.results), res


def kernel(x, labels, weight, bias):
    out, _ = run(x, labels, weight, bias, trace=False)
    return out
